# revision 2
# baseline (speedup 1.0000x reference)
"""Trainium2 Bass kernel for nn_CalAttenMap (gnn message passing + dense softmax).

Strategy (8 fully independent NeuronCores, NO collectives):
  - Each core owns output rows [256c, 256(c+1)).  Host merges duplicate
    (a,b) pairs, appends diag entries, groups edges per 128-row block and
    pre-transposes the union stream to bf16.
  - No AllGather: every core computes the o-projection for ALL 2048 nodes
    on its own PE (bf16) into an SBUF row-table; s-projection for its own
    256 rows only.  This removes the cross-core launch-skew serialization
    the collective version suffered (~145 us on the profiled core).
  - Per-edge o rows come from an SBUF-source dma_gather (transpose mode,
    tokens_per_rank=128): SBUF->SBUF, zero HBM traffic, output arrives
    pre-transposed c-major [128, KC, e].  s rows are gathered by one-hot
    matmul on PE (K=128, reusing the one-hot built for the softmax tail).
  - Everything bf16: union stream, gather tables, prod GEMM operands,
    dense fill, and the scatter-add payload (error budget ~0.5% << 2e-2).
  - Softmax identity: out = 1/Z on background cells, exp(af)/Z on edges,
    0 on diag; Z(i,h) = (N-1-cnt_i) + sum_edges exp(af).  Fill broadcasts
    recip (bf16); dma_scatter_add applies edge deltas as 256B tokens of 8
    cells (cell//8 fits int16 exactly).
  - The SDMA scatter-add LOSES concurrent adds to the same 256B row within
    one call (verified on HW), so the host colors edges into phases with
    unique cell//8 tokens per (group, phase); phase calls alternate between
    the two row-groups so same-group phases never have in-flight overlap,
    and a barrier orders each group's dense fill before its first RMW phase.
  - Exact-zero diagonal: rg (the gathered recip) is matmul'd from the SAME
    bf16 recip used for the fill, so fill + (0 - rg) cancels exactly.
"""

import numpy as np
import ml_dtypes

import concourse.bass as bass
import concourse.bacc as bacc
import concourse.mybir as mybir
import concourse.tile as tile
from concourse import bass_utils
from concourse.bass_interp import get_hw_module

F32 = mybir.dt.float32
BF16 = mybir.dt.bfloat16
I16 = mybir.dt.int16
BF = ml_dtypes.bfloat16

N = 2048          # nodes
D = 512           # feature dim
P = 16            # heads per pair
N_CORES = 8
R = N // N_CORES  # rows per core (256)
G = R // 128      # row groups per core (2)
KC = D // 128     # contraction chunks (4)
MBE = 512         # edges per macroblock

LAST_RESULTS = None


# --------------------------------------------------------------------------
# host prep
# --------------------------------------------------------------------------

def _host_prep(obj_feats, union_feats, pair_idxs):
    a = pair_idxs[:, 0].astype(np.int64)
    b = pair_idxs[:, 1].astype(np.int64)
    key = a * N + b
    order = np.argsort(key, kind="stable")
    ks = key[order]
    uniq_mask = np.ones(len(ks), bool)
    if len(ks) > 1:
        uniq_mask[1:] = ks[1:] != ks[:-1]
    starts = np.nonzero(uniq_mask)[0]
    ku = ks[starts]
    mult = np.diff(np.append(starts, len(ks))).astype(np.float32)
    u_merged = np.add.reduceat(union_feats[order], starts, axis=0).astype(np.float32)

    au = ku // N
    bu = ku % N
    gate = (au != bu)

    cnt = np.bincount(au[gate], minlength=N)
    base = (N - 1 - cnt).astype(np.float32)

    # diagonal entries only for rows WITHOUT a self-edge
    has_self = np.zeros(N, bool)
    has_self[au[~gate]] = True
    diag = np.nonzero(~has_self)[0].astype(np.int64)
    A = np.concatenate([au, diag])
    B_ = np.concatenate([bu, diag])
    LG = np.concatenate([np.where(gate, 0.0, -30000.0).astype(np.float32),
                         np.full(len(diag), -30000.0, np.float32)])
    MU = np.concatenate([mult, np.zeros(len(diag), np.float32)])
    PM = np.ones(len(A), np.float32)       # 1 for real/diag, 0 for padding
    UI = np.concatenate([np.arange(len(ku), dtype=np.int64),
                         np.full(len(diag), -1)])
    o2 = np.argsort(A, kind="stable")
    A, B_, LG, MU, PM, UI = A[o2], B_[o2], LG[o2], MU[o2], PM[o2], UI[o2]

    n_groups = N // 128
    gidx = A // 128
    tok8 = ((A % 128) * N + B_) // 8

    # scatter-add loses concurrent same-row (cell//8) adds within one call:
    # color edges into phases with unique tokens per (group, phase).
    per_group = []
    for t in range(n_groups):
        remaining = np.nonzero(gidx == t)[0]
        phases = []
        while len(remaining):
            toks = tok8[remaining]
            first = np.zeros(len(remaining), bool)
            _, fi = np.unique(toks, return_index=True)
            first[fi] = True
            phases.append(remaining[first])
            remaining = remaining[~first]
        per_group.append(phases)
    PH = max(len(p) for p in per_group)
    BBs = []
    for p in range(PH):
        mx = max((len(pg[p]) if p < len(pg) else 0) for pg in per_group)
        BBs.append(max(1, int(-(-mx // 128))))
    while sum(BBs) % 4:
        BBs[-1] += 1
    BB = sum(BBs)          # blocks per 128-row group
    cap = BB * 128

    e_tot = n_groups * cap
    Ap = np.empty(e_tot, np.int64)
    Bp = np.zeros(e_tot, np.int64)
    LGp = np.full(e_tot, -30000.0, np.float32)
    MUp = np.zeros(e_tot, np.float32)
    PMp = np.zeros(e_tot, np.float32)
    UIp = np.full(e_tot, -1, np.int64)
    for t in range(n_groups):
        used = set(tok8[gidx == t].tolist())
        ftok = next(x for x in range(N * 128 // 8) if x not in used)
        a_pad = t * 128 + (ftok * 8) // N
        b_pad = (ftok * 8) % N
        d0 = t * cap
        off = 0
        for p in range(PH):
            lst = (per_group[t][p] if p < len(per_group[t])
                   else np.array([], np.int64))
            n0 = len(lst)
            sl = slice(d0 + off, d0 + off + n0)
            Ap[sl] = A[lst]
            Bp[sl] = B_[lst]
            LGp[sl] = LG[lst]
            MUp[sl] = MU[lst]
            PMp[sl] = PM[lst]
            UIp[sl] = UI[lst]
            pad = slice(d0 + off + n0, d0 + off + BBs[p] * 128)
            Ap[pad] = a_pad
            Bp[pad] = b_pad
            off += BBs[p] * 128

    nb = G * BB
    MB = nb // 4          # macroblocks per core
    E = nb * 128          # padded edges per core
    per_core = []
    for c in range(N_CORES):
        sl = slice(c * G * cap, (c + 1) * G * cap)
        Ac, Bc, LGc, MUc, PMc, UIc = (Ap[sl], Bp[sl], LGp[sl], MUp[sl],
                                      PMp[sl], UIp[sl])
        union_c = np.zeros((E, D), np.float32)
        has_u = UIc >= 0
        union_c[has_u] = u_merged[UIc[has_u]]
        # uT[mb][p][c*MBE+e] = u[e, c*128+p]  (c-major)
        uT = np.ascontiguousarray(
            union_c.reshape(MB, MBE, KC, 128).transpose(0, 3, 2, 1)
            .reshape(MB, 128, KC * MBE)).astype(BF)
        # gather idxs for o rows (wrap 16, replicate x8)
        bp = np.ascontiguousarray(
            np.tile(Bc.reshape(MB, MBE // 16, 16).transpose(0, 2, 1), (1, 8, 1))
            .transpose(1, 0, 2).reshape(128, MB * (MBE // 16)).astype(np.int16))
        cell = (Ac % 128) * N + Bc
        sidx = (cell // 8).astype(np.int16)
        scp = np.ascontiguousarray(
            np.tile(sidx.reshape(nb * 8, 16).T, (8, 1)))  # [128, nb*8]
        per_core.append(dict(
            uT=uT,
            b_pack=bp,
            a_row=np.ascontiguousarray(
                (Ac % 128).astype(np.float32).reshape(1, E)),
            a_relT=np.ascontiguousarray(
                (Ac % 128).astype(BF).reshape(nb, 128).T),
            bg=np.ascontiguousarray(np.stack([MUc, LGc]).astype(BF)),
            cmod8T=np.ascontiguousarray(
                (cell % 8).astype(BF).reshape(nb, 128).T),
            pmT=np.ascontiguousarray(PMc.astype(np.float32).reshape(nb, 128).T),
            sc_pack=scp,
            base=np.ascontiguousarray(base[c * R:(c + 1) * R].reshape(G, 128).T),
            objTo=np.ascontiguousarray(
                obj_feats[c * R:(c + 1) * R].T.reshape(KC, 128, R)
                .transpose(1, 0, 2).reshape(128, KC * R)).astype(BF),
        ))
    return BBs, per_core


# --------------------------------------------------------------------------
# device program
# --------------------------------------------------------------------------

def _build_program(BBs):
    BB = sum(BBs)
    PH = len(BBs)
    nc = bacc.Bacc("TRN2", target_bir_lowering=False, debug=False,
                   enable_asserts=True, num_devices=N_CORES)

    nb = G * BB
    MB = nb // 4          # macroblocks per core
    MBG = MB // G         # macroblocks per group
    E = nb * 128

    dt_in = lambda name, shape, dt=F32: nc.dram_tensor(
        name, shape, dt, kind="ExternalInput").ap()

    objT_full = dt_in("objT_full", [128, KC * N], BF16)
    objTo = dt_in("objTo", [128, KC * R], BF16)
    wsT = dt_in("wsT", [128, KC * D], BF16)
    woT = dt_in("woT", [128, KC * D], BF16)
    wwT = dt_in("wwT", [128, KC * P], BF16)
    wsb = dt_in("wsb", [1, D], BF16)
    wob = dt_in("wob", [1, D], BF16)
    ones = dt_in("ones", [1, 128], BF16)
    bg_lhs = dt_in("bg_lhs", [2, P], BF16)
    colidx = dt_in("colidx", [128, 128], BF16)
    rowidx = dt_in("rowidx", [128, 1])
    ident16 = dt_in("ident16", [16, 16])
    base_d = dt_in("base", [128, G])
    uT_d = dt_in("uT", [MB, 128, KC * MBE], BF16)
    b_pack = dt_in("b_pack", [128, MB * (MBE // 16)], I16)
    a_row = dt_in("a_row", [1, E])
    a_relT = dt_in("a_relT", [128, nb], BF16)
    bg_d = dt_in("bg", [2, E], BF16)
    cmod8T = dt_in("cmod8T", [128, nb], BF16)
    pmT = dt_in("pmT", [128, nb])
    sc_pack = dt_in("sc_pack", [128, nb * 8], I16)
    iota8 = dt_in("iota8", [128, 8], BF16)

    out_ds = [nc.dram_tensor(f"out{g}", [128 * N, P], BF16,
                             kind="ExternalOutput").ap()
              for g in range(G)]

    eq = mybir.AluOpType.is_equal
    SUB = mybir.AluOpType.subtract
    EXP = mybir.ActivationFunctionType.Exp

    with tile.TileContext(nc) as tc:
        with tc.tile_pool(name="const", bufs=1) as cp:
            wwT_sb = cp.tile([128, KC * P], BF16)
            nc.sync.dma_start(wwT_sb[:], wwT[:])
            bgl_sb = cp.tile([2, P], BF16)
            nc.sync.dma_start(bgl_sb[:], bg_lhs[:])
            colidx_sb = cp.tile([128, 128], BF16)
            nc.sync.dma_start(colidx_sb[:], colidx[:])
            rowidx_sb = cp.tile([128, 1], F32)
            nc.sync.dma_start(rowidx_sb[:], rowidx[:])
            id16_sb = cp.tile([16, 16], F32)
            nc.sync.dma_start(id16_sb[:], ident16[:])
            base_sb = cp.tile([128, G], F32)
            nc.sync.dma_start(base_sb[:], base_d[:])
            bpack_sb = cp.tile([128, MB * (MBE // 16)], I16)
            nc.sync.dma_start(bpack_sb[:], b_pack[:])
            a_relT_sb = cp.tile([128, nb], BF16)
            nc.sync.dma_start(a_relT_sb[:], a_relT[:])
            cmod8_sb = cp.tile([128, nb], BF16)
            nc.sync.dma_start(cmod8_sb[:], cmod8T[:])
            pm_sb = cp.tile([128, nb], F32)
            nc.sync.dma_start(pm_sb[:], pmT[:])
            scp_sb = cp.tile([128, nb * 8], I16)
            nc.sync.dma_start(scp_sb[:], sc_pack[:])
            iota8_sb = cp.tile([128, 8], BF16)
            nc.sync.dma_start(iota8_sb[:], iota8[:])
            o_rows = cp.tile([128, (N // 128) * D], BF16)   # row n -> part n%128, stripe n//128
            s_rows = cp.tile([128, G * D], BF16)            # own row r -> part r%128, stripe r//128

            # ---------------- stage A: projections ----------------
            with tc.tile_pool(name="aconst", bufs=1) as ac, \
                 tc.tile_pool(name="proj_psum", bufs=2, space="PSUM") as pp:
                objT_sb = ac.tile([128, KC * N], BF16)
                nc.sync.dma_start(objT_sb[:], objT_full[:])
                objTo_sb = ac.tile([128, KC * R], BF16)
                nc.sync.dma_start(objTo_sb[:], objTo[:])
                wsT_sb = ac.tile([128, KC * D], BF16)
                nc.sync.dma_start(wsT_sb[:], wsT[:])
                woT_sb = ac.tile([128, KC * D], BF16)
                nc.sync.dma_start(woT_sb[:], woT[:])
                wsb_sb = ac.tile([1, D], BF16)
                nc.sync.dma_start(wsb_sb[:], wsb[:])
                wob_sb = ac.tile([1, D], BF16)
                nc.sync.dma_start(wob_sb[:], wob[:])
                ones_sb = ac.tile([1, 128], BF16)
                nc.sync.dma_start(ones_sb[:], ones[:])
                for j in range(N // 128):
                    pt = pp.tile([128, D], F32)
                    for kc in range(KC):
                        nc.tensor.matmul(
                            pt[:],
                            lhsT=objT_sb[:, kc * N + j * 128: kc * N + (j + 1) * 128],
                            rhs=woT_sb[:, kc * D:(kc + 1) * D],
                            start=(kc == 0), stop=False)
                    nc.tensor.matmul(pt[:], lhsT=ones_sb[:1, :], rhs=wob_sb[:1, :],
                                     start=False, stop=True)
                    nc.scalar.copy(o_rows[:, j * D:(j + 1) * D], pt[:])
                for g in range(G):
                    pt = pp.tile([128, D], F32)
                    for kc in range(KC):
                        nc.tensor.matmul(
                            pt[:],
                            lhsT=objTo_sb[:, kc * R + g * 128: kc * R + (g + 1) * 128],
                            rhs=wsT_sb[:, kc * D:(kc + 1) * D],
                            start=(kc == 0), stop=False)
                    nc.tensor.matmul(pt[:], lhsT=ones_sb[:1, :], rhs=wsb_sb[:1, :],
                                     start=False, stop=True)
                    nc.scalar.copy(s_rows[:, g * D:(g + 1) * D], pt[:])

            # ---------------- stage B ----------------
            with tc.tile_pool(name="st_psum", bufs=2, space="PSUM") as stp, \
                 tc.tile_pool(name="af_psum", bufs=1, space="PSUM") as afp, \
                 tc.tile_pool(name="tr_psum", bufs=1, space="PSUM") as trp, \
                 tc.tile_pool(name="zmb_psum", bufs=1, space="PSUM") as zp, \
                 tc.tile_pool(name="rg_psum", bufs=1, space="PSUM") as rgp, \
                 tc.tile_pool(name="work", bufs=3) as wk, \
                 tc.tile_pool(name="grpbuf", bufs=2) as gb, \
                 tc.tile_pool(name="bigwork", bufs=3) as bw, \
                 tc.tile_pool(name="fillp", bufs=2) as fp_:
                st_g = {}

                def loop_group(g):
                    zT_run = gb.tile([16, 128], F32, tag="zt")
                    nc.vector.memset(zT_run[:], 0.0)
                    expf_g = gb.tile([128, BB * P], BF16, tag="expf")
                    ohTf_g = gb.tile([128, BB * 128], BF16, tag="ohTf")
                    st_g[g] = (zT_run, expf_g, ohTf_g)
                    arb_g = gb.tile([128, MBG * MBE], F32, tag="arb")
                    nc.sync.dma_start(
                        arb_g[:],
                        a_row[0:1, g * MBG * MBE:(g + 1) * MBG * MBE]
                        .to_broadcast((128, MBG * MBE)))
                    bg_g = gb.tile([2, MBG * MBE], BF16, tag="bgm")
                    nc.sync.dma_start(bg_g[:],
                                      bg_d[:, g * MBG * MBE:(g + 1) * MBG * MBE])
                    for m in range(MBG):
                        mb = g * MBG + m
                        uT_t = bw.tile([128, KC * MBE], BF16, tag="uT")
                        nc.sync.dma_start(uT_t[:], uT_d[mb, :, :])
                        oT_g = bw.tile([128, KC * MBE], BF16, tag="oTg")
                        nc.gpsimd.dma_gather(
                            oT_g[:].rearrange("p (c e) -> p c e", c=KC),
                            o_rows[:],
                            bpack_sb[:, mb * (MBE // 16):(mb + 1) * (MBE // 16)],
                            MBE, MBE, elem_size=D, transpose=True,
                            sbuf_tokens_per_rank=128,
                            sbuf_free_dim_per_rank=2 * D)
                        ohT_sl = ohTf_g[:, m * MBE:(m + 1) * MBE]
                        nc.vector.tensor_tensor(
                            out=ohT_sl,
                            in0=rowidx_sb[:, 0:1].to_broadcast([128, MBE]),
                            in1=arb_g[:, m * MBE:(m + 1) * MBE], op=eq)
                        t_t = bw.tile([128, KC * MBE], BF16, tag="tt")
                        for c in range(KC):
                            st_ps = stp.tile([128, MBE], F32, tag="st")
                            nc.tensor.matmul(
                                st_ps[:],
                                lhsT=s_rows[:, g * D + c * 128: g * D + (c + 1) * 128],
                                rhs=ohT_sl, start=True, stop=True)
                            nc.vector.tensor_mul(
                                t_t[:, c * MBE:(c + 1) * MBE], st_ps[:],
                                oT_g[:, c * MBE:(c + 1) * MBE])
                        prodT = bw.tile([128, KC * MBE], BF16, tag="prod")
                        nc.vector.tensor_mul(prodT[:], t_t[:], uT_t[:])
                        af_ps = afp.tile([16, MBE], F32)
                        for c in range(KC):
                            nc.tensor.matmul(
                                af_ps[:],
                                lhsT=wwT_sb[:, c * P:(c + 1) * P],
                                rhs=prodT[:, c * MBE:(c + 1) * MBE],
                                start=(c == 0), stop=False)
                        nc.tensor.matmul(af_ps[:], lhsT=bgl_sb[:, :],
                                         rhs=bg_g[:, m * MBE:(m + 1) * MBE],
                                         start=False, stop=True)
                        expfT_t = wk.tile([16, MBE], F32, tag="expfT")
                        nc.scalar.activation(expfT_t[:], af_ps[:], EXP)
                        etr_ps = trp.tile([128, 4 * P], F32)
                        for q in range(4):
                            nc.tensor.transpose(
                                etr_ps[:, q * P:(q + 1) * P],
                                expfT_t[:, q * 128:(q + 1) * 128], id16_sb[:])
                        nc.scalar.copy(
                            expf_g[:, (m * 4) * P:(m * 4 + 4) * P], etr_ps[:])
                        zmb_ps = zp.tile([16, 128], F32, tag="zmb")
                        for q in range(4):
                            blk = g * BB + m * 4 + q
                            kk = m * 4 + q
                            oh_t = wk.tile([128, 128], BF16, tag="oh")
                            nc.vector.tensor_tensor(
                                out=oh_t[:],
                                in0=a_relT_sb[:, blk:blk + 1].to_broadcast([128, 128]),
                                in1=colidx_sb[:], op=eq)
                            nc.tensor.matmul(
                                zmb_ps[:], lhsT=expf_g[:, kk * P:(kk + 1) * P],
                                rhs=oh_t[:],
                                start=(q == 0), stop=(q == 3))
                        nc.vector.tensor_add(zT_run[:], zT_run[:], zmb_ps[:])

                def tail_group(g):
                    zT_run, expf_g, ohTf_g = st_g[g]
                    ztr_ps = trp.tile([128, 4 * P], F32)
                    nc.tensor.transpose(ztr_ps[:, :P], zT_run[:], id16_sb[:])
                    z_t = wk.tile([128, P], F32, tag="z")
                    nc.vector.tensor_scalar_add(z_t[:], ztr_ps[:, :P],
                                                base_sb[:, g:g + 1])
                    recip_t = wk.tile([128, P], F32, tag="recip")
                    nc.vector.reciprocal(recip_t[:], z_t[:])
                    recip_bf = wk.tile([128, P], BF16, tag="recipbf")
                    nc.vector.tensor_copy(recip_bf[:], recip_t[:])
                    fill_t = fp_.tile([128, 128 * P], BF16, tag="fill")
                    nc.vector.tensor_copy(
                        fill_t[:].rearrange("p (j q) -> p j q", q=P),
                        recip_bf[:, None, :].broadcast_to([128, 128, P]))
                    out4 = out_ds[g].rearrange("(r a j) q -> r a j q",
                                               a=N // 128, j=128)
                    nc.sync.dma_start(
                        out4[:, :, :, :],
                        fill_t[:].rearrange("p (j q) -> p j q", q=P)[:, None, :, :]
                        .broadcast_to([128, N // 128, 128, P]))
                    val8_g = gb.tile([128, BB * 128], BF16, tag="val8")
                    rgall_ps = rgp.tile([128, BB * P], F32)
                    for k in range(BB):
                        nc.tensor.matmul(
                            rgall_ps[:, k * P:(k + 1) * P],
                            lhsT=ohTf_g[:, k * 128:(k + 1) * 128],
                            rhs=recip_bf[:], start=True, stop=True)
                    for k in range(BB):
                        blk = g * BB + k
                        rg_ps = rgall_ps[:, k * P:(k + 1) * P]
                        val_t = wk.tile([128, P], F32, tag="val")
                        nc.vector.tensor_mul(val_t[:], expf_g[:, k * P:(k + 1) * P],
                                             rg_ps[:])
                        rgm_t = wk.tile([128, P], F32, tag="rgm")
                        nc.vector.tensor_scalar_mul(rgm_t[:], rg_ps[:],
                                                    pm_sb[:, blk:blk + 1])
                        dl_t = wk.tile([128, P], F32, tag="dl")
                        nc.vector.tensor_tensor(out=dl_t[:], in0=val_t[:],
                                                in1=rgm_t[:], op=SUB)
                        soh_t = wk.tile([128, 8], BF16, tag="soh")
                        nc.vector.tensor_tensor(
                            out=soh_t[:],
                            in0=cmod8_sb[:, blk:blk + 1].to_broadcast([128, 8]),
                            in1=iota8_sb[:], op=eq)
                        nc.vector.tensor_tensor(
                            out=val8_g[:, k * 128:(k + 1) * 128]
                            .rearrange("p (s q) -> p s q", s=8),
                            in0=soh_t[:, :, None].broadcast_to([128, 8, P]),
                            in1=dl_t[:, None, :].broadcast_to([128, 8, P]),
                            op=mybir.AluOpType.mult)
                    return val8_g

                offs = [0]
                for bbp in BBs:
                    offs.append(offs[-1] + bbp)

                def scatter_group(g, val8_g, p):
                    off, bbp = offs[p], BBs[p]
                    out128 = out_ds[g].rearrange("(c f) p -> c (f p)", f=8)
                    nc.gpsimd.dma_scatter_add(
                        out128[:, :],
                        val8_g[:, off * 128:(off + bbp) * 128]
                        .rearrange("p (k q) -> p k q", q=128),
                        scp_sb[:, (g * BB + off) * 8:(g * BB + off + bbp) * 8],
                        bbp * 128, bbp * 128, elem_size=128)

                # order: loop0, tail0(+fill0), loop1 (fill0 drains under it);
                # then a barrier-separated ladder of scatter phases -- a
                # barrier between consecutive phases of the same group keeps
                # same-row RMWs from racing (fill included via first barrier).
                loop_group(0)
                v0 = tail_group(0)
                loop_group(1)
                tc.strict_bb_all_engine_barrier()
                scatter_group(0, v0, 0)
                v1 = tail_group(1)
                tc.strict_bb_all_engine_barrier()
                # alternate groups: the intervening call separates same-group
                # phases (the SWDGE call occupies Q7 until its drain), so no
                # further barriers are needed.
                for p in range(1, PH + 1):
                    scatter_group(1, v1, p - 1)
                    if p < PH:
                        scatter_group(0, v0, p)

    nc.compile()
    return nc


# --------------------------------------------------------------------------
# entry point
# --------------------------------------------------------------------------

def kernel(obj_feats, union_feats, pair_idxs, ws_w, ws_b, wo_w, wo_b, w_w, w_b):
    global LAST_RESULTS
    obj_feats = np.asarray(obj_feats, np.float32)
    union_feats = np.asarray(union_feats, np.float32)
    pair_idxs = np.asarray(pair_idxs)
    ws_w = np.asarray(ws_w, np.float32)
    wo_w = np.asarray(wo_w, np.float32)
    w_w = np.asarray(w_w, np.float32)

    BB, per_core = _host_prep(obj_feats, union_feats, pair_idxs)
    nc = _build_program(BB)

    def chunkT(M, cols):   # [K, cols] -> [128, KC*cols] with [p, kc*cols+n] = M[kc*128+p, n]
        K = M.shape[0]
        return np.ascontiguousarray(
            M.reshape(K // 128, 128, cols).transpose(1, 0, 2)
            .reshape(128, (K // 128) * cols))

    shared = dict(
        objT_full=chunkT(obj_feats.T, N).astype(BF),
        wsT=chunkT(np.ascontiguousarray(ws_w.T), D).astype(BF),
        woT=chunkT(np.ascontiguousarray(wo_w.T), D).astype(BF),
        wwT=chunkT(np.ascontiguousarray(w_w.T), P).astype(BF),
        wsb=np.asarray(ws_b, np.float32).reshape(1, D).astype(BF),
        wob=np.asarray(wo_b, np.float32).reshape(1, D).astype(BF),
        ones=np.ones((1, 128), BF),
        bg_lhs=np.ascontiguousarray(
            np.stack([np.asarray(w_b, np.float32),
                      np.ones(P, np.float32)])).astype(BF),
        colidx=np.broadcast_to(np.arange(128, dtype=np.float32),
                               (128, 128)).astype(BF).copy(),
        rowidx=np.arange(128, dtype=np.float32).reshape(128, 1),
        iota8=np.broadcast_to(np.arange(8, dtype=np.float32),
                              (128, 8)).astype(BF).copy(),
        ident16=np.eye(16, dtype=np.float32),
    )
    in_maps = []
    for c in range(N_CORES):
        pc = per_core[c]
        in_maps.append({
            **shared,
            "objTo": pc["objTo"],
            "base": pc["base"],
            "uT": pc["uT"],
            "b_pack": pc["b_pack"],
            "a_row": pc["a_row"],
            "a_relT": pc["a_relT"],
            "bg": pc["bg"],
            "cmod8T": pc["cmod8T"],
            "pmT": pc["pmT"],
            "sc_pack": pc["sc_pack"],
        })

    nc.m = get_hw_module(nc.m)
    res = bass_utils.run_bass_kernel_spmd(nc, in_maps, core_ids=list(range(N_CORES)))
    LAST_RESULTS = res

    out = np.empty((N, N, P), np.float32)
    for c in range(N_CORES):
        for g in range(G):
            out[c * R + g * 128: c * R + (g + 1) * 128] = \
                np.asarray(res.results[c][f"out{g}"]).astype(np.float32).reshape(128, N, P)
    return out
